# revision 2
# baseline (speedup 1.0000x reference)
"""Trainium2 Bass kernel for nn_ATSearchKNN (retrieval_knn).

Strategy: the reference computation is fully per-graph independent (the
AdaptiveBlending MLPs are pointwise, batch_normalize uses per-graph stats,
and the KNN is masked to same-graph candidates; `batch` is sorted so each
graph is a contiguous row range).  With B=8 graphs and 8 NeuronCores we
assign one graph per core.  Each core, fully on device:

  - builds feat/pos feature rows in transposed layout [features, points]
    (the 32-dim positional encoding, pure sin/cos of the inputs, is
    precomputed on host because the ACT engine's Sin table has no range
    reduction and our arguments reach |x|~140)
  - runs both tiny MLPs + softmax blending (PE matmuls + ACT/DVE)
  - per-graph mean/unbiased-std normalization (DVE reductions)
  - scores s[i,j] = 2*z_i.z_j - sq_j - sq_i  (= -d2) via PE fp32 matmuls
    with the -sq_j and padding mask folded in as a rank-1 accumulation
  - exact top-32 per query row via DVE max8/max_index/match_replace
    (matches jax.lax.top_k tie-breaking: descending value, ascending index)

Host only does: input slicing/padding per graph, the sin/cos encoding
table, weight-row permutation (to avoid interleaved partition writes),
and the final index gather/offset.
"""

import math
import numpy as np

NCORES = 8
K = 32
NFREQ_USED = 16  # enc truncated to 32 = 16 sin + 16 cos of x-coordinate
MAXFREQ = 10.0
NFREQ = 64
BIG = 3.0e38

_PROGRAM_CACHE = {}


def _build_program(NP):
    """Build the (SPMD-shared) Bass program for padded per-core size NP."""
    import concourse.bass as bass  # noqa: F401
    import concourse.mybir as mybir
    import concourse.tile as tile
    from concourse import bacc

    AF = mybir.ActivationFunctionType
    ALU = mybir.AluOpType
    AX = mybir.AxisListType
    f32 = mybir.dt.float32
    u32 = mybir.dt.uint32

    NT = NP // 128
    # candidate-axis chunks for matmuls (PSUM bank = 512 fp32)
    chunks = []
    c0 = 0
    while c0 < NP:
        cw = min(512, NP - c0)
        chunks.append((c0, cw))
        c0 += cw

    nc = bacc.Bacc("TRN2", num_devices=NCORES, debug=False)

    # ---- DRAM I/O ----
    d_encT = nc.dram_tensor("encT", [32, NP], f32, kind="ExternalInput")
    d_fstat = nc.dram_tensor("fstat", [17, NP], f32, kind="ExternalInput")
    d_pstat = nc.dram_tensor("pstat", [4, NP], f32, kind="ExternalInput")
    d_maskbig = nc.dram_tensor("maskbig", [1, NP], f32, kind="ExternalInput")
    d_maskval = nc.dram_tensor("maskval", [1, NP], f32, kind="ExternalInput")
    d_scal = nc.dram_tensor("scal", [128, 4], f32, kind="ExternalInput")
    d_w1f = nc.dram_tensor("w1f_aug", [49, 32], f32, kind="ExternalInput")
    d_w2f = nc.dram_tensor("w2f_aug", [33, 1], f32, kind="ExternalInput")
    d_w1p = nc.dram_tensor("w1p_aug", [36, 32], f32, kind="ExternalInput")
    d_w2p = nc.dram_tensor("w2p_aug", [33, 1], f32, kind="ExternalInput")
    d_idx = nc.dram_tensor("idx_out", [NT, 128, K], u32, kind="ExternalOutput")

    with tile.TileContext(nc) as tc:
        with (
            tc.tile_pool(name="big", bufs=1) as big,
            tc.tile_pool(name="rows", bufs=1) as rows,
            tc.tile_pool(name="small", bufs=1) as small,
            tc.tile_pool(name="scorep", bufs=2) as scorep,
            tc.tile_pool(name="idxp", bufs=2) as idxp,
            tc.tile_pool(name="mxp", bufs=2) as mxp,
        ):
            # ---- persistent SBUF tiles ----
            featT = big.tile([49, NP], f32, tag="featT")
            peT = big.tile([36, NP], f32, tag="peT")
            h1fT = big.tile([33, NP], f32, tag="h1fT")
            h1pT = big.tile([33, NP], f32, tag="h1pT")
            zT = big.tile([99, NP], f32, tag="zT")
            zsT = big.tile([99, NP], f32, tag="zsT")  # z^2 scratch, then 2*z

            fwT = rows.tile([1, NP], f32, tag="fwT")
            pwT = rows.tile([1, NP], f32, tag="pwT")
            mrow = rows.tile([1, NP], f32, tag="mrow")
            srow = rows.tile([1, NP], f32, tag="srow")
            sqrow = rows.tile([1, NP], f32, tag="sqrow")
            sqmrow = rows.tile([1, NP], f32, tag="sqmrow")
            mbig = rows.tile([1, NP], f32, tag="mbig")
            mval = rows.tile([1, NP], f32, tag="mval")

            w1f = small.tile([49, 32], f32, tag="w1f")
            w2f = small.tile([33, 1], f32, tag="w2f")
            w1p = small.tile([36, 32], f32, tag="w1p")
            w2p = small.tile([33, 1], f32, tag="w2p")
            scal = small.tile([128, 4], f32, tag="scal")
            ones99 = small.tile([99, 1], f32, tag="ones99")
            onesrow = small.tile([1, 128], f32, tag="onesrow")
            negrow = small.tile([1, 128], f32, tag="negrow")
            ident1 = small.tile([1, 1], f32, tag="ident1")
            negsqP = small.tile([128, NT], f32, tag="negsqP")
            st_sum = small.tile([99, 1], f32, tag="st_sum")
            st_ssq = small.tile([99, 1], f32, tag="st_ssq")
            st_mean = small.tile([99, 1], f32, tag="st_mean")
            st_nm2 = small.tile([99, 1], f32, tag="st_nm2")
            st_var = small.tile([99, 1], f32, tag="st_var")
            st_std = small.tile([99, 1], f32, tag="st_std")
            st_rstd = small.tile([99, 1], f32, tag="st_rstd")

            # ---- load inputs ----
            nc.sync.dma_start(out=featT[0:32, :], in_=d_encT.ap())
            nc.sync.dma_start(out=featT[32:49, :], in_=d_fstat.ap())
            nc.sync.dma_start(out=peT[0:32, :], in_=d_encT.ap())
            nc.sync.dma_start(out=peT[32:36, :], in_=d_pstat.ap())
            nc.sync.dma_start(out=mbig, in_=d_maskbig.ap())
            nc.sync.dma_start(out=mval, in_=d_maskval.ap())
            nc.sync.dma_start(out=scal, in_=d_scal.ap())
            nc.sync.dma_start(out=w1f, in_=d_w1f.ap())
            nc.sync.dma_start(out=w2f, in_=d_w2f.ap())
            nc.sync.dma_start(out=w1p, in_=d_w1p.ap())
            nc.sync.dma_start(out=w2p, in_=d_w2p.ap())

            nc.vector.memset(ones99, 1.0)
            nc.vector.memset(onesrow, 1.0)
            nc.vector.memset(negrow, -1.0)
            nc.vector.memset(ident1, 1.0)
            nc.vector.memset(h1fT[32:33, :], 1.0)
            nc.vector.memset(h1pT[32:33, :], 1.0)
            # zT rows 48..63 are zero-padding between the two blocks
            nc.vector.memset(zT[32:64, :], 0.0)

            # ============ phase A: MLPs / softmax / z / sq (PSUM pools) ====
            with (
                tc.tile_pool(name="ps_mlp", bufs=2, space="PSUM") as ps_mlp,
                tc.tile_pool(name="ps_row", bufs=2, space="PSUM") as ps_row,
                tc.tile_pool(name="ps_sw", bufs=1, space="PSUM") as ps_sw,
                tc.tile_pool(name="ps_tr", bufs=2, space="PSUM") as ps_tr,
            ):
                # MLP layer 1 + relu (both branches), then layer 2
                for c0, cw in chunks:
                    sl = slice(c0, c0 + cw)
                    pf = ps_mlp.tile([32, 512], f32, tag="pmlp")
                    nc.tensor.matmul(pf[:, :cw], lhsT=w1f, rhs=featT[:, sl],
                                     start=True, stop=True)
                    nc.scalar.activation(h1fT[0:32, sl], pf[:, :cw], AF.Relu)
                    pp = ps_mlp.tile([32, 512], f32, tag="pmlp")
                    nc.tensor.matmul(pp[:, :cw], lhsT=w1p, rhs=peT[:, sl],
                                     start=True, stop=True)
                    nc.scalar.activation(h1pT[0:32, sl], pp[:, :cw], AF.Relu)
                for c0, cw in chunks:
                    sl = slice(c0, c0 + cw)
                    pw1 = ps_row.tile([1, 512], f32, tag="prow")
                    nc.tensor.matmul(pw1[:, :cw], lhsT=w2f, rhs=h1fT[:, sl],
                                     start=True, stop=True)
                    nc.scalar.activation(fwT[:, sl], pw1[:, :cw], AF.Copy)
                    pw2 = ps_row.tile([1, 512], f32, tag="prow")
                    nc.tensor.matmul(pw2[:, :cw], lhsT=w2p, rhs=h1pT[:, sl],
                                     start=True, stop=True)
                    nc.scalar.activation(pwT[:, sl], pw2[:, :cw], AF.Copy)

                # softmax over the 2 logits (row layout); exp/scale in place
                nc.vector.tensor_tensor(out=mrow, in0=fwT, in1=pwT, op=ALU.max)
                nc.vector.tensor_sub(fwT, fwT, mrow)
                nc.vector.tensor_sub(pwT, pwT, mrow)
                nc.scalar.activation(fwT, fwT, AF.Exp)
                nc.scalar.activation(pwT, pwT, AF.Exp)
                nc.vector.tensor_add(srow, fwT, pwT)
                nc.vector.reciprocal(srow, srow)
                nc.vector.tensor_mul(fwT, fwT, srow)   # fwT = swf
                nc.vector.tensor_mul(pwT, pwT, srow)   # pwT = swp
                # zero padded points so they don't pollute the statistics
                nc.vector.tensor_mul(fwT, fwT, mval)
                nc.vector.tensor_mul(pwT, pwT, mval)

                # combined = [feat*swf ; 0pad ; pe*swp] in zT
                for c0, cw in chunks:
                    sl = slice(c0, c0 + cw)
                    bf = ps_sw.tile([48, 512], f32, tag="bswf")
                    nc.tensor.matmul(bf[:, :cw], lhsT=onesrow[0:1, 0:48],
                                     rhs=fwT[:, sl], start=True, stop=True)
                    bp = ps_sw.tile([35, 512], f32, tag="bswp")
                    nc.tensor.matmul(bp[:, :cw], lhsT=onesrow[0:1, 0:35],
                                     rhs=pwT[:, sl], start=True, stop=True)
                    nc.vector.tensor_mul(zT[0:32, sl], featT[0:32, sl],
                                         bf[0:32, :cw])
                    nc.vector.tensor_mul(zT[32:48, sl], featT[32:48, sl],
                                         bf[32:48, :cw])
                    nc.vector.tensor_mul(zT[64:96, sl], peT[0:32, sl],
                                         bp[0:32, :cw])
                    nc.vector.tensor_mul(zT[96:99, sl], peT[32:35, sl],
                                         bp[32:35, :cw])

                # per-graph normalization
                nc.vector.reduce_sum(st_sum, zT, axis=AX.X)
                nc.vector.tensor_mul(zsT, zT, zT)
                nc.vector.reduce_sum(st_ssq, zsT, axis=AX.X)
                nc.vector.tensor_scalar_mul(st_mean, st_sum, scal[0:99, 1:2])
                nc.vector.tensor_mul(st_nm2, st_mean, st_mean)
                nc.vector.tensor_scalar_mul(st_nm2, st_nm2, scal[0:99, 0:1])
                nc.vector.tensor_sub(st_var, st_ssq, st_nm2)
                nc.vector.tensor_scalar_mul(st_var, st_var, scal[0:99, 2:3])
                nc.vector.tensor_scalar_max(st_var, st_var, 0.0)
                nc.scalar.activation(st_std, st_var, AF.Sqrt)
                nc.vector.tensor_scalar_add(st_std, st_std, 1e-8)
                nc.vector.reciprocal(st_rstd, st_std)
                # z = (combined - mean) * rstd   (one fused pass)
                nc.vector.tensor_scalar(zT, zT, st_mean[:, 0:1], st_rstd[:, 0:1],
                                        op0=ALU.subtract, op1=ALU.mult)

                # squared norms sq_j (row layout) + mask
                nc.vector.tensor_mul(zsT, zT, zT)
                for c0, cw in chunks:
                    sl = slice(c0, c0 + cw)
                    pq = ps_row.tile([1, 512], f32, tag="prow")
                    nc.tensor.matmul(pq[:, :cw], lhsT=ones99, rhs=zsT[:, sl],
                                     start=True, stop=True)
                    nc.scalar.activation(sqrow[:, sl], pq[:, :cw], AF.Copy)
                nc.vector.tensor_add(sqmrow, sqrow, mbig)

                # -sq_i per query partition (PE transpose per tile)
                for t in range(NT):
                    ptr = ps_tr.tile([128, 1], f32, tag="ptr")
                    nc.tensor.transpose(ptr, sqrow[0:1, 128 * t:128 * (t + 1)],
                                        ident1)
                    nc.scalar.activation(negsqP[:, t:t + 1], ptr, AF.Copy,
                                         scale=-1.0)

                # 2*z for the stationary operand (after sq matmuls read zsT)
                nc.vector.tensor_add(zsT, zT, zT)

            # ============ phase B: distance scores + exact top-32 ==========
            with tc.tile_pool(name="ps_sc", bufs=4, space="PSUM") as ps_sc:
                for t in range(NT):
                    qsl = slice(128 * t, 128 * (t + 1))
                    sc = scorep.tile([128, NP], f32, tag="sc")
                    for c0, cw in chunks:
                        sl = slice(c0, c0 + cw)
                        ps = ps_sc.tile([128, 512], f32, tag="psc")
                        nc.tensor.matmul(ps[:, :cw], lhsT=zsT[:, qsl],
                                         rhs=zT[:, sl], start=True, stop=False)
                        nc.tensor.matmul(ps[:, :cw], lhsT=negrow,
                                         rhs=sqmrow[:, sl], start=False,
                                         stop=True)
                        nc.scalar.activation(sc[:, sl], ps[:, :cw], AF.Identity,
                                             bias=negsqP[:, t:t + 1])
                    idxt = idxp.tile([128, K], u32, tag="idxt")
                    for g in range(4):
                        mx = mxp.tile([128, 8], f32, tag="mx")
                        nc.vector.max(mx, sc)
                        nc.vector.max_index(idxt[:, 8 * g:8 * g + 8], mx, sc)
                        if g < 3:
                            nc.vector.match_replace(out=sc, in_to_replace=mx,
                                                    in_values=sc,
                                                    imm_value=-BIG)
                    nc.sync.dma_start(out=d_idx.ap()[t], in_=idxt)

    nc.compile()
    return nc


def _host_prep(x, pos, batch, w1f, b1f, w2f, b2f, w1p, b1p, w2p, b2p):
    """Shard per graph, build per-core input maps (all host work is O(N*F))."""
    batch_i = np.asarray(batch).astype(np.int64)
    sizes = np.bincount(batch_i, minlength=NCORES).astype(np.int64)
    offs = np.concatenate([[0], np.cumsum(sizes)])
    NP = max(128, int(math.ceil(sizes.max() / 128.0)) * 128)

    # frequency bands (match reference: linspace(1, MAXFREQ, NFREQ) first 16)
    fb = np.linspace(1.0, MAXFREQ, NFREQ).astype(np.float32)[:NFREQ_USED]

    # permuted+augmented weights (feature order: sin16, cos16, x/xyz, bias)
    w1f = np.asarray(w1f, dtype=np.float32)
    w1p = np.asarray(w1p, dtype=np.float32)
    sin_rows_f = 16 + 2 * np.arange(16)
    cos_rows_f = 17 + 2 * np.arange(16)
    w1f_aug = np.concatenate(
        [w1f[sin_rows_f], w1f[cos_rows_f], w1f[0:16],
         np.asarray(b1f, np.float32)[None, :]], axis=0)
    sin_rows_p = 3 + 2 * np.arange(16)
    cos_rows_p = 4 + 2 * np.arange(16)
    w1p_aug = np.concatenate(
        [w1p[sin_rows_p], w1p[cos_rows_p], w1p[0:3],
         np.asarray(b1p, np.float32)[None, :]], axis=0)
    w2f_aug = np.concatenate(
        [np.asarray(w2f, np.float32), np.asarray(b2f, np.float32)[None, :]],
        axis=0)
    w2p_aug = np.concatenate(
        [np.asarray(w2p, np.float32), np.asarray(b2p, np.float32)[None, :]],
        axis=0)

    in_maps = []
    for b in range(NCORES):
        n = int(sizes[b])
        sl = slice(int(offs[b]), int(offs[b + 1]))
        xg = np.zeros((NP, 16), np.float32)
        xg[:n] = np.asarray(x[sl], np.float32)
        pg = np.zeros((NP, 3), np.float32)
        pg[:n] = np.asarray(pos[sl], np.float32)

        x0 = pg[:, 0]
        xf = x0[:, None] * fb[None, :]
        encT = np.concatenate([np.sin(xf).T, np.cos(xf).T],
                              axis=0).astype(np.float32)

        fstat = np.concatenate([xg.T, np.ones((1, NP), np.float32)], axis=0)
        pstat = np.concatenate([pg.T, np.ones((1, NP), np.float32)], axis=0)
        maskbig = np.zeros((1, NP), np.float32)
        maskbig[0, n:] = BIG
        maskval = np.zeros((1, NP), np.float32)
        maskval[0, :n] = 1.0
        scal = np.zeros((128, 4), np.float32)
        scal[:, 0] = np.float32(n)
        scal[:, 1] = np.float32(1.0) / np.float32(max(n, 1))
        scal[:, 2] = np.float32(1.0) / np.float32(max(n - 1, 1))

        in_maps.append({
            "encT": np.ascontiguousarray(encT),
            "fstat": np.ascontiguousarray(fstat),
            "pstat": np.ascontiguousarray(pstat),
            "maskbig": maskbig,
            "maskval": maskval,
            "scal": scal,
            "w1f_aug": np.ascontiguousarray(w1f_aug),
            "w2f_aug": np.ascontiguousarray(w2f_aug),
            "w1p_aug": np.ascontiguousarray(w1p_aug),
            "w2p_aug": np.ascontiguousarray(w2p_aug),
        })
    return in_maps, sizes, offs, NP


def kernel(x, pos, batch, w1f, b1f, w2f, b2f, w1p, b1p, w2p, b2p):
    from concourse import bass_utils

    in_maps, sizes, offs, NP = _host_prep(
        x, pos, batch, w1f, b1f, w2f, b2f, w1p, b1p, w2p, b2p)

    if NP not in _PROGRAM_CACHE:
        _PROGRAM_CACHE[NP] = _build_program(NP)
    nc = _PROGRAM_CACHE[NP]

    res = bass_utils.run_bass_kernel_spmd(
        nc, in_maps, core_ids=list(range(NCORES)))

    N = x.shape[0]
    out_dtype = np.asarray(batch).dtype
    col_parts = []
    for b in range(NCORES):
        n = int(sizes[b])
        idx = res.results[b]["idx_out"].reshape(-1, K)[:n].astype(np.int64)
        col_parts.append(idx + int(offs[b]))
    col = np.concatenate(col_parts, axis=0).reshape(-1).astype(out_dtype)
    row = np.repeat(np.arange(N, dtype=np.int64), K).astype(out_dtype)
    return row, col
